# revision 40
# baseline (speedup 1.0000x reference)
"""Int8 per-token-quantized linear (MluQuantLinearInt8) on 8 Trainium2 cores.

  out[s, n] = (sum_k q[s,k] * w[n,k]) * x_scale[s] * w_scale[n]
  q = round(x / x_scale) clipped to [-127, 127],  x_scale = max(|x|_row, 1e-8)/127

Sharding: data-parallel over tokens (512/core); weights replicated, streamed
once per core. The GEMM runs at the 16-bit PE roofline (~885us for 4096
matmuls/core), so all recoverable time is pre-GEMM; the startup pipeline is
built around three hardware constraints measured from NTFF traces:
(1) HW-queue DMAs round-robin 8 serial semaphore lanes in scheduler order,
(2) transfers in flight share ~350GB/s, so any early weight byte starves the
quant-critical x stream, (3) each engine is a FIFO, so a descriptor gated on
a far-future event blocks everything behind it on that engine.

Structure:
  - x streams half-tile-at-a-time on the Scalar queue (that engine runs only
    x loads + activations); per-nt weight tiles (1.05MB) ride the Sync queue
    (first 6) and the software DGE (rest), paced by quant progress/pool
    rotation so only nt0 gates the first matmul.
  - quantization never materializes q: the scalar engine emits
    y = fp16(x*inv + 1536) (fp16 ulp is exactly 1.0 on [1024,2048), so the
    conversion RNE-rounds to integer), y is XBAR-transposed to qT, and the
    GEMM runs fp16 x fp16 on the +1536-offset values - exact in fp32 PSUM.
    The eviction folds the offset back out with the host-precomputed
    -1536*rowsum(w) (exact in f32: 3*rs*2^9, |3*rs| < 2^24), then applies
    w_scale and x_scale: two DVE ops per output tile.
  - the GEMM starts early at half token-width: output tiles nt0-3 run
    tokens 0-255 as soon as token tiles 0/1 are transposed (range-based
    slice dependencies), while tiles 2/3 still quantize.
  - junk warm-up matmuls gated on quant-pipeline events keep the PE HAM
    activity window busy so the real GEMM starts at 2.4 GHz instead of 1.2.

Measured: 951-957us vs 964us for the pre-restructure baseline; absmax-
relative error 5.1e-4 (tolerance 2e-3).
"""

import sys
from contextlib import ExitStack
from functools import lru_cache

import numpy as np

for _p in ("/opt/trn_rl_repo", "/root/.axon_site/_ro/trn_rl_repo"):
    if _p not in sys.path:
        sys.path.append(_p)

import ml_dtypes  # noqa: E402

import concourse.bass as bass  # noqa: E402
import concourse.bass2jax as bass2jax  # noqa: E402
import concourse.mybir as mybir  # noqa: E402
import concourse.tile as tile  # noqa: E402
from concourse.bass_utils import (  # noqa: E402
    compile_bir_kernel as _orig_compile_bir_kernel,
    run_bass_kernel_spmd,
)
from concourse.masks import make_identity  # noqa: E402

# The walrus build in this container accepts only ONE sync-wait per
# instruction ("Too many sync wait commands", CoreV3GenImpl setupSyncWait) —
# Tile's kernel-tail drain carries several. Split extra waits onto preceding
# single-wait EventSemaphore carriers on the same engine (engine program order
# makes the AND of waits equivalent).
import json as _json  # noqa: E402


def _split_multi_waits(bir_json):
    d = _json.loads(bir_json)
    changed = False
    for fn in d.get("functions", []):
        for bb in fn.get("blocks", []) or []:
            insts = bb.get("instructions")
            if not insts:
                continue
            out = []
            for ins in insts:
                si = ins.get("sync_info")
                waits = (si or {}).get("on_wait") or []
                if len(waits) > 1:
                    for j, w in enumerate(waits[:-1]):
                        out.append(
                            {
                                "engine": ins.get("engine"),
                                "ins": [],
                                "outs": [],
                                "name": f"{ins.get('name', 'I')}_w{j}",
                                "opcode": "EventSemaphore",
                                "sync_info": {"on_update": [], "on_wait": [w]},
                            }
                        )
                    si["on_wait"] = [waits[-1]]
                    changed = True
                out.append(ins)
            bb["instructions"] = out
    if not changed:
        return bir_json
    return _json.dumps(d).encode()


def _patched_compile_bir_kernel(bir_json, tmpdir, neff_name="file.neff"):
    return _orig_compile_bir_kernel(
        _split_multi_waits(bir_json), tmpdir, neff_name=neff_name
    )


bass2jax.compile_bir_kernel = _patched_compile_bir_kernel

P = 128
NCORES = 8
S, K_FULL, N_FULL = 4096, 4096, 16384
QMAX = 127.0
# fp16 ulp is exactly 1.0 on [1024, 2048): converting x*inv + 1536 to fp16
# RNE-rounds to integer; y - 1536 recovers q (|q| <= 127.5 keeps y inside
# [1408, 1664) ⊂ [1024, 2048)).
MAGIC16 = 1536.0
F32 = mybir.dt.float32
BF16 = mybir.dt.bfloat16
FP16 = mybir.dt.float16

WBUFS = 6  # per-nt weight tiles resident in SBUF (first WBUFS prequeued)
NT_EARLY = 4  # leading output tiles run at half token-width


def build_nc(S_C, K, N, warmup=True):
    """One-core program; SPMD-replicated across cores by the runner.

    Inputs (per core):
      x   [S_C, K]  f32 - this core's token slice
      wt  [NT, P, KC, P] bf16 - weights, host-packed per output-channel tile
      ws  [P, NT]   f32 - weight_scale packed ws[p, nt] = weight_scale[nt*128+p]
    Output:
      outT [N, S_C] f32 - dequantized output, transposed
    """
    KC = K // P  # contraction chunks
    KH = K // 2  # K half
    KCH = KC // 2
    TT = S_C // P  # token tiles
    SH = S_C // 2  # token half (per-core)
    NT = N // P  # output-channel tiles (one psum tile each)

    nc = bass.Bass()
    x = nc.declare_dram_parameter("x", [S_C, K], F32, isOutput=False)
    wt = nc.declare_dram_parameter("wt", [NT, P, KC, P], FP16, isOutput=False)
    wsb = nc.declare_dram_parameter("wsb", [P, 2 * NT], F32, isOutput=False)
    outT = nc.declare_dram_parameter("outT", [N, S_C], F32, isOutput=True)
    xs_scratch = nc.dram_tensor("xs_scratch", [S_C], F32)

    outT_t = outT.rearrange("(nt p) s -> nt p s", p=P)
    # xs_scratch[t*128 + p] viewed as [p, t]: partition-major write target
    xs_pt = xs_scratch.rearrange("(t p) -> p t", p=P)

    with tile.TileContext(nc) as tc, ExitStack() as ctx:
        const_pool = ctx.enter_context(tc.tile_pool(name="const", bufs=1))
        xpool = ctx.enter_context(tc.tile_pool(name="xp", bufs=4))
        ypool = ctx.enter_context(tc.tile_pool(name="yp", bufs=3))
        qt_pool = ctx.enter_context(tc.tile_pool(name="qt", bufs=1))
        wpool = ctx.enter_context(tc.tile_pool(name="wp", bufs=WBUFS))
        opool = ctx.enter_context(tc.tile_pool(name="op", bufs=4))
        spool = ctx.enter_context(tc.tile_pool(name="sp", bufs=1))
        ps_pool = ctx.enter_context(tc.tile_pool(name="psp", bufs=2, space="PSUM"))
        ph_pool = ctx.enter_context(tc.tile_pool(name="php", bufs=2, space="PSUM"))
        pt_pool = ctx.enter_context(tc.tile_pool(name="ptp", bufs=1, space="PSUM"))
        pw_pool = ctx.enter_context(tc.tile_pool(name="pwp", bufs=1, space="PSUM"))
        ptr_pool = ctx.enter_context(tc.tile_pool(name="ptrp", bufs=2, space="PSUM"))

        ident_f32 = const_pool.tile([P, P], F32)
        make_identity(nc, ident_f32)
        ident_fp16 = const_pool.tile([P, P], FP16)
        nc.vector.tensor_copy(ident_fp16, ident_f32)

        wsb_sb = const_pool.tile([P, 2 * NT], F32)
        nc.gpsimd.dma_start(wsb_sb, wsb[:, :])

        # ---- x half-tile loads: one sequential HBM stream on the Scalar
        # queue, ahead of every weight byte, so arrivals stagger and the
        # quant pipeline starts as early as possible.
        xts = []
        for t in range(TT):
            xt = xpool.tile([P, K], F32, name=f"xt{t}", tag="xt")
            for h in range(2):
                nc.scalar.dma_start(
                    xt[:, h * KH : (h + 1) * KH],
                    x[t * P : (t + 1) * P, h * KH : (h + 1) * KH],
                )
            xts.append(xt)

        wtiles = {}
        amaxes = [
            spool.tile([P, 1], F32, name=f"amax{t}", tag=f"amax{t}")
            for t in range(TT)
        ]

        def preload_w(nt, gate):
            # The Tile scheduler orders ungated DMAs first, and transfers in
            # flight share HBM bandwidth - so an early weight descriptor
            # starves the quant-critical x stream. Gate each preload behind
            # quant progress via a WAW dep: a tiny gpsimd write into the
            # weight buffer that reads `gate`.
            wtile = wpool.tile([P, KC, P], FP16, name=f"wt{nt}", tag="wtile")
            nc.gpsimd.tensor_copy(wtile[:, 0, :1], gate)
            nc.sync.dma_start(wtile, wt[nt])
            wtiles[nt] = wtile

        # ---- Phase 1: per-token dynamic int8 quantization + transpose ----
        # qT[k%128, t, k//128, tok%128]: each transpose target is
        # per-partition contiguous (non-contiguous dst breaks DMA transpose)
        qT = qt_pool.tile([P, TT, KC, P], FP16)
        xs_all = spool.tile([P, TT], F32)  # xs_all[p, t] = x_scale[t*128+p]
        xsb = spool.tile([P, S_C], F32, tag="xsb")

        wu_ps = (
            pw_pool.tile([P, 512], F32, name="wu_ps", tag="wu_ps") if warmup else None
        )

        def quant_tile(t):
            xt = xts[t]
            amh = [
                spool.tile([P, 1], F32, name=f"amh{t}{h}", tag=f"amh{t}{h}")
                for h in range(2)
            ]
            for h in range(2):
                nc.vector.tensor_reduce(
                    out=amh[h],
                    in_=xt[:, h * KH : (h + 1) * KH],
                    axis=mybir.AxisListType.X,
                    op=mybir.AluOpType.max,
                    apply_absolute_value=True,
                )
            amax = amaxes[t]
            nc.vector.tensor_tensor(
                out=amax, in0=amh[0], in1=amh[1], op=mybir.AluOpType.max
            )
            # amax' = max(amax, 1e-8); x_scale = amax'/127 (~1ulp, via *1/127);
            # q = round(x * (127 * recip(amax'))) - DVE has no divide, but
            # reciprocal is bit-exact; the ~1ulp quantizer error flips a
            # rounding boundary on ~0.1 elements per 4096-row (negligible).
            nc.vector.tensor_scalar(amax, amax, 1e-8, None, op0=mybir.AluOpType.max)
            nc.vector.tensor_scalar(
                xs_all[:, t : t + 1],
                amax,
                float(np.float32(1.0 / 127.0)),
                None,
                op0=mybir.AluOpType.mult,
            )
            inv = spool.tile([P, 1], F32, name=f"inv{t}", tag=f"inv{t}")
            nc.vector.reciprocal(inv, amax)
            nc.vector.tensor_scalar(inv, inv, QMAX, None, op0=mybir.AluOpType.mult)

            y = ypool.tile([P, K], FP16, name=f"y{t}", tag="y")
            for h in range(2):
                hs = slice(h * KH, (h + 1) * KH)
                # y = fp16(x*inv + 1536) on the scalar engine: the fp16
                # conversion RNE-rounds to integer. The GEMM runs directly on
                # the offset values; the eviction subtracts 1536*rowsum(w).
                nc.scalar.activation(
                    y[:, hs],
                    xt[:, hs],
                    mybir.ActivationFunctionType.Copy,
                    bias=MAGIC16,
                    scale=inv,
                )
                if warmup and t < 2:
                    nc.tensor.matmul(
                        wu_ps, lhsT=ident_f32, rhs=xt[:, h * KH : h * KH + 512]
                    )
                if warmup and (t < 2 or (t == 2 and h == 0)):
                    nc.tensor.matmul(
                        wu_ps, lhsT=ident_fp16, rhs=y[:, h * KH : h * KH + 512]
                    )
                if t < 2:
                    # t0/t1 transpose on the PE (128x128 blocks into PSUM,
                    # copied out per block): no XBAR DMA, so no DMA-transpose
                    # barrier forms against the early weight stream, and the
                    # blocks hold the HAM clock warm through startup.
                    for kc in range(h * KCH, (h + 1) * KCH):
                        ptr = ptr_pool.tile(
                            [P, P], FP16, name=f"ptr{t}_{kc}", tag="ptr"
                        )
                        nc.tensor.transpose(
                            ptr, y[:, kc * P : (kc + 1) * P], ident_fp16
                        )
                        # gpsimd cannot read PSUM; t0 evacuates via the DVE,
                        # t1 via scalar activation-copies so the two tiles'
                        # copy streams run on different engines.
                        if t == 0:
                            nc.vector.tensor_copy(qT[:, t, kc, :], ptr)
                        else:
                            nc.scalar.activation(
                                qT[:, t, kc, :], ptr,
                                mybir.ActivationFunctionType.Copy,
                            )
                else:
                    # half-tile transpose on the DMA xbar (Sync queue):
                    # [tok, (kc ki)] -> [ki, kc, tok]
                    nc.sync.dma_start(
                        qT[:, t, h * KCH : (h + 1) * KCH, :], y[:, hs],
                        transpose=True,
                    )

        xs_ps = pt_pool.tile([2, 2 * P], F32, name="xs_ps", tag="xs_ps")

        def xs_pair(pair):
            # x_scale for token tiles (2*pair, 2*pair+1): PE-transpose to a
            # [t, tok] layout (contiguous 512B dram rows), round-trip through
            # DRAM on the Scalar queue to broadcast [tok] across partitions.
            ts = slice(2 * pair, 2 * pair + 2)
            cols = slice(pair * P, (pair + 1) * P)
            nc.tensor.transpose(xs_ps[:, cols], xs_all[:, ts], ident_f32)
            xs_row = spool.tile([2, P], F32, name=f"xs_row{pair}", tag=f"xs_row{pair}")
            nc.vector.tensor_copy(xs_row, xs_ps[:, cols])
            nc.sync.dma_start(
                xs_scratch.rearrange("(t p) -> t p", p=P)[ts, :], xs_row
            )
            nc.sync.dma_start(
                xsb[:, pair * SH : (pair + 1) * SH],
                xs_scratch[None, pair * SH : (pair + 1) * SH].to_broadcast((P, SH)),
            )

        def gemm_tile(nt, half):
            # half=None: full 512-token width; 0/1: 256-token half
            if nt in wtiles:
                wtile = wtiles[nt]
            else:
                wtile = wpool.tile([P, KC, P], FP16, name=f"wt{nt}", tag="wtile")
                nc.gpsimd.dma_start(wtile, wt[nt])
                wtiles[nt] = wtile
            if half is None:
                ps = ps_pool.tile([P, S_C], F32, name=f"ps{nt}", tag="ps")
                rhs_t = slice(0, TT)
                cols = slice(0, S_C)
                xs_in = xsb
            else:
                ps = ph_pool.tile([P, SH], F32, name=f"ph{nt}_{half}", tag="ph")
                rhs_t = slice(2 * half, 2 * half + 2)
                cols = slice(half * SH, (half + 1) * SH)
                xs_in = xsb[:, cols]
            for kc in range(KC):
                nc.tensor.matmul(
                    ps,
                    lhsT=wtile[:, kc, :],
                    rhs=qT[:, rhs_t, kc, :],
                    start=(kc == 0),
                    stop=(kc == KC - 1),
                )
            out_sb = opool.tile([P, S_C], F32, name=f"o{nt}_{half}", tag="out_sb")
            # acc = sum_k (q+1536)*w; (acc - 1536*rowsum_n) * ws_n * xs_tok
            nc.vector.tensor_scalar(
                out_sb[:, cols],
                ps,
                wsb_sb[:, NT + nt : NT + nt + 1],
                wsb_sb[:, nt : nt + 1],
                op0=mybir.AluOpType.add,
                op1=mybir.AluOpType.mult,
            )
            nc.vector.tensor_tensor(
                out=out_sb[:, cols], in0=out_sb[:, cols], in1=xs_in,
                op=mybir.AluOpType.mult,
            )
            nc.sync.dma_start(outT_t[nt][:, cols], out_sb[:, cols])

        # DMA issue order matters: HW-queue DMAs round-robin over 8 serial
        # semaphore lanes in issue order, so a DMA can end up waiting on the
        # DMA 8 slots earlier. Keep the order (x*8, tr-tile0/1*4, w0-2,
        # xs0, tr-tile2/3*4 interleaved, w3-5, xs1) so every lane pairing is
        # early-completing -> early-gated.
        quant_tile(0)
        quant_tile(1)
        for nt in range(3):
            preload_w(nt, amaxes[2])
        xs_pair(0)
        quant_tile(2)
        quant_tile(3)
        for nt in range(3, WBUFS):
            preload_w(nt, amaxes[3])

        # ---- Phase 2: streamed weights-stationary GEMM + fused dequant ----
        # leading tiles at half width (tokens 0-255) overlap the tail of the
        # quant pipeline; their other half + the rest run at full width.
        for nt in range(NT_EARLY):
            gemm_tile(nt, half=0)
        xs_pair(1)
        for nt in range(NT_EARLY):
            gemm_tile(nt, half=1)
        for nt in range(NT_EARLY, NT):
            gemm_tile(nt, half=None)

    return nc


def pack_inputs(input_tensor, weight, weight_scale, S_C, K, N):
    """Host-side prep: shard x, pack weights to bf16 per-nt SBUF layout."""
    KC = K // P
    NT = N // P
    x = np.ascontiguousarray(input_tensor.reshape(-1, K))  # [S, K]
    w16 = weight.astype(np.float16)  # [N, K], int8 values exact
    # pack[nt, p, kc, j] = w[nt*128 + j, kc*128 + p]
    wt = np.ascontiguousarray(w16.reshape(NT, P, KC, P).transpose(0, 3, 2, 1))
    ws = weight_scale.reshape(NT, P).T.astype(np.float32)  # [P, NT]
    # -1536 * rowsum(w): 3*rs*2^9 with |3*rs| < 2^24, exact in f32
    rs = weight.astype(np.int64).sum(axis=1).astype(np.float32)
    wb = (np.float32(-1536.0) * rs).reshape(NT, P).T
    wsb = np.ascontiguousarray(np.concatenate([ws, wb], axis=1))  # [P, 2*NT]
    return x, wt, wsb


@lru_cache(maxsize=2)
def _compiled_nc(S_C, K, N, warmup):
    return build_nc(S_C, K, N, warmup=warmup)


def run(input_tensor, weight, weight_scale, n_cores=NCORES, trace=False,
        exact_divide=True, warmup=True):
    Sfull, K = input_tensor.shape[-2], input_tensor.shape[-1]
    N = weight.shape[0]
    S_C = Sfull // n_cores
    x, wt, wsb = pack_inputs(input_tensor, weight, weight_scale, S_C, K, N)
    nc = _compiled_nc(S_C, K, N, warmup)
    in_maps = [
        {"x": np.ascontiguousarray(x[c * S_C : (c + 1) * S_C]),
         "wt": wt, "wsb": wsb}
        for c in range(n_cores)
    ]
    res = run_bass_kernel_spmd(nc, in_maps, core_ids=list(range(n_cores)), trace=trace)
    out = np.empty((Sfull, N), np.float32)
    for c in range(n_cores):
        out[c * S_C : (c + 1) * S_C] = res.results[c]["outT"].T
    return out[None], res


def kernel(input_tensor, weight, weight_scale):
    out, _ = run(
        np.asarray(input_tensor), np.asarray(weight), np.asarray(weight_scale)
    )
    return out



# revision 41
# speedup vs baseline: 1.0072x; 1.0072x over previous
"""Int8 per-token-quantized linear (MluQuantLinearInt8) on 8 Trainium2 cores.

  out[s, n] = (sum_k q[s,k] * w[n,k]) * x_scale[s] * w_scale[n]
  q = round(x / x_scale) clipped to [-127, 127],  x_scale = max(|x|_row, 1e-8)/127

Sharding: data-parallel over tokens (512/core); weights replicated, streamed
once per core. The GEMM runs at the 16-bit PE roofline (~885us for 4096
matmuls/core), so all recoverable time is pre-GEMM; the startup pipeline is
built around three hardware constraints measured from NTFF traces:
(1) HW-queue DMAs round-robin 8 serial semaphore lanes in scheduler order,
(2) transfers in flight share ~350GB/s, so any early weight byte starves the
quant-critical x stream, (3) each engine is a FIFO, so a descriptor gated on
a far-future event blocks everything behind it on that engine.

Structure:
  - x streams half-tile-at-a-time on the Scalar queue (that engine runs only
    x loads + activations); per-nt weight tiles (1.05MB) ride the Sync queue
    (first 6) and the software DGE (rest), paced by quant progress/pool
    rotation so only nt0 gates the first matmul.
  - quantization never materializes q: the scalar engine emits
    y = fp16(x*inv + 1536) (fp16 ulp is exactly 1.0 on [1024,2048), so the
    conversion RNE-rounds to integer), y is XBAR-transposed to qT, and the
    GEMM runs fp16 x fp16 on the +1536-offset values - exact in fp32 PSUM.
    The eviction folds the offset back out with the host-precomputed
    -1536*rowsum(w) (exact in f32: 3*rs*2^9, |3*rs| < 2^24), then applies
    w_scale and x_scale: two DVE ops per output tile.
  - the GEMM starts early at half token-width: output tiles nt0-3 run
    tokens 0-255 as soon as token tiles 0/1 are transposed (range-based
    slice dependencies), while tiles 2/3 still quantize.
  - junk warm-up matmuls gated on quant-pipeline events keep the PE HAM
    activity window busy so the real GEMM starts at 2.4 GHz instead of 1.2.

Measured: 951-957us vs 964us for the pre-restructure baseline; absmax-
relative error 5.1e-4 (tolerance 2e-3).
"""

import sys
from contextlib import ExitStack
from functools import lru_cache

import numpy as np

for _p in ("/opt/trn_rl_repo", "/root/.axon_site/_ro/trn_rl_repo"):
    if _p not in sys.path:
        sys.path.append(_p)

import ml_dtypes  # noqa: E402

import concourse.bass as bass  # noqa: E402
import concourse.bass2jax as bass2jax  # noqa: E402
import concourse.mybir as mybir  # noqa: E402
import concourse.tile as tile  # noqa: E402
from concourse.bass_utils import (  # noqa: E402
    compile_bir_kernel as _orig_compile_bir_kernel,
    run_bass_kernel_spmd,
)
from concourse.masks import make_identity  # noqa: E402

# The walrus build in this container accepts only ONE sync-wait per
# instruction ("Too many sync wait commands", CoreV3GenImpl setupSyncWait) —
# Tile's kernel-tail drain carries several. Split extra waits onto preceding
# single-wait EventSemaphore carriers on the same engine (engine program order
# makes the AND of waits equivalent).
import json as _json  # noqa: E402


def _split_multi_waits(bir_json):
    d = _json.loads(bir_json)
    changed = False
    for fn in d.get("functions", []):
        for bb in fn.get("blocks", []) or []:
            insts = bb.get("instructions")
            if not insts:
                continue
            out = []
            for ins in insts:
                si = ins.get("sync_info")
                waits = (si or {}).get("on_wait") or []
                if len(waits) > 1:
                    for j, w in enumerate(waits[:-1]):
                        out.append(
                            {
                                "engine": ins.get("engine"),
                                "ins": [],
                                "outs": [],
                                "name": f"{ins.get('name', 'I')}_w{j}",
                                "opcode": "EventSemaphore",
                                "sync_info": {"on_update": [], "on_wait": [w]},
                            }
                        )
                    si["on_wait"] = [waits[-1]]
                    changed = True
                out.append(ins)
            bb["instructions"] = out
    if not changed:
        return bir_json
    return _json.dumps(d).encode()


def _patched_compile_bir_kernel(bir_json, tmpdir, neff_name="file.neff"):
    return _orig_compile_bir_kernel(
        _split_multi_waits(bir_json), tmpdir, neff_name=neff_name
    )


bass2jax.compile_bir_kernel = _patched_compile_bir_kernel

P = 128
NCORES = 8
S, K_FULL, N_FULL = 4096, 4096, 16384
QMAX = 127.0
# fp16 ulp is exactly 1.0 on [1024, 2048): converting x*inv + 1536 to fp16
# RNE-rounds to integer; y - 1536 recovers q (|q| <= 127.5 keeps y inside
# [1408, 1664) ⊂ [1024, 2048)).
MAGIC16 = 1536.0
F32 = mybir.dt.float32
BF16 = mybir.dt.bfloat16
FP16 = mybir.dt.float16

WBUFS = 6  # per-nt weight tiles resident in SBUF (first WBUFS prequeued)
NT_EARLY = 4  # leading output tiles run at half token-width


def build_nc(S_C, K, N, warmup=True):
    """One-core program; SPMD-replicated across cores by the runner.

    Inputs (per core):
      x   [S_C, K]  f32 - this core's token slice
      wt  [NT, P, KC, P] bf16 - weights, host-packed per output-channel tile
      ws  [P, NT]   f32 - weight_scale packed ws[p, nt] = weight_scale[nt*128+p]
    Output:
      outT [N, S_C] f32 - dequantized output, transposed
    """
    KC = K // P  # contraction chunks
    KH = K // 2  # K half
    KCH = KC // 2
    TT = S_C // P  # token tiles
    SH = S_C // 2  # token half (per-core)
    NT = N // P  # output-channel tiles (one psum tile each)

    nc = bass.Bass()
    x = nc.declare_dram_parameter("x", [S_C, K], F32, isOutput=False)
    wt = nc.declare_dram_parameter("wt", [NT, P, KC, P], FP16, isOutput=False)
    wsb = nc.declare_dram_parameter("wsb", [P, 2 * NT], F32, isOutput=False)
    outT = nc.declare_dram_parameter("outT", [N, S_C], F32, isOutput=True)
    xs_scratch = nc.dram_tensor("xs_scratch", [S_C], F32)

    outT_t = outT.rearrange("(nt p) s -> nt p s", p=P)
    # xs_scratch[t*128 + p] viewed as [p, t]: partition-major write target
    xs_pt = xs_scratch.rearrange("(t p) -> p t", p=P)

    with tile.TileContext(nc) as tc, ExitStack() as ctx:
        const_pool = ctx.enter_context(tc.tile_pool(name="const", bufs=1))
        xpool = ctx.enter_context(tc.tile_pool(name="xp", bufs=4))
        ypool = ctx.enter_context(tc.tile_pool(name="yp", bufs=3))
        qt_pool = ctx.enter_context(tc.tile_pool(name="qt", bufs=1))
        wpool = ctx.enter_context(tc.tile_pool(name="wp", bufs=WBUFS))
        opool = ctx.enter_context(tc.tile_pool(name="op", bufs=4))
        spool = ctx.enter_context(tc.tile_pool(name="sp", bufs=1))
        ps_pool = ctx.enter_context(tc.tile_pool(name="psp", bufs=4, space="PSUM"))
        ph_pool = ctx.enter_context(tc.tile_pool(name="php", bufs=2, space="PSUM"))
        pt_pool = ctx.enter_context(tc.tile_pool(name="ptp", bufs=1, space="PSUM"))
        pw_pool = ctx.enter_context(tc.tile_pool(name="pwp", bufs=1, space="PSUM"))

        ident_f32 = const_pool.tile([P, P], F32)
        make_identity(nc, ident_f32)
        ident_fp16 = const_pool.tile([P, P], FP16)
        nc.vector.tensor_copy(ident_fp16, ident_f32)

        wsb_sb = const_pool.tile([P, 2 * NT], F32)
        nc.gpsimd.dma_start(wsb_sb, wsb[:, :])

        # ---- x half-tile loads: one sequential HBM stream on the Scalar
        # queue, ahead of every weight byte, so arrivals stagger and the
        # quant pipeline starts as early as possible.
        xts = []
        for t in range(TT):
            xt = xpool.tile([P, K], F32, name=f"xt{t}", tag="xt")
            for h in range(2):
                nc.scalar.dma_start(
                    xt[:, h * KH : (h + 1) * KH],
                    x[t * P : (t + 1) * P, h * KH : (h + 1) * KH],
                )
            xts.append(xt)

        wtiles = {}
        amaxes = [
            spool.tile([P, 1], F32, name=f"amax{t}", tag=f"amax{t}")
            for t in range(TT)
        ]

        def preload_w(nt, gate):
            # The Tile scheduler orders ungated DMAs first, and transfers in
            # flight share HBM bandwidth - so an early weight descriptor
            # starves the quant-critical x stream. Gate each preload behind
            # quant progress via a WAW dep: a tiny gpsimd write into the
            # weight buffer that reads `gate`.
            wtile = wpool.tile([P, KC, P], FP16, name=f"wt{nt}", tag="wtile")
            nc.gpsimd.tensor_copy(wtile[:, 0, :1], gate)
            nc.sync.dma_start(wtile, wt[nt])
            wtiles[nt] = wtile

        # ---- Phase 1: per-token dynamic int8 quantization + transpose ----
        # qT[k%128, t, k//128, tok%128]: each transpose target is
        # per-partition contiguous (non-contiguous dst breaks DMA transpose)
        qT = qt_pool.tile([P, TT, KC, P], FP16)
        xs_all = spool.tile([P, TT], F32)  # xs_all[p, t] = x_scale[t*128+p]
        xsb = spool.tile([P, S_C], F32, tag="xsb")

        wu_ps = (
            pw_pool.tile([P, 512], F32, name="wu_ps", tag="wu_ps") if warmup else None
        )

        def quant_tile(t):
            xt = xts[t]
            amh = [
                spool.tile([P, 1], F32, name=f"amh{t}{h}", tag=f"amh{t}{h}")
                for h in range(2)
            ]
            for h in range(2):
                nc.vector.tensor_reduce(
                    out=amh[h],
                    in_=xt[:, h * KH : (h + 1) * KH],
                    axis=mybir.AxisListType.X,
                    op=mybir.AluOpType.max,
                    apply_absolute_value=True,
                )
            amax = amaxes[t]
            nc.vector.tensor_tensor(
                out=amax, in0=amh[0], in1=amh[1], op=mybir.AluOpType.max
            )
            # amax' = max(amax, 1e-8); x_scale = amax'/127 (~1ulp, via *1/127);
            # q = round(x * (127 * recip(amax'))) - DVE has no divide, but
            # reciprocal is bit-exact; the ~1ulp quantizer error flips a
            # rounding boundary on ~0.1 elements per 4096-row (negligible).
            nc.vector.tensor_scalar(amax, amax, 1e-8, None, op0=mybir.AluOpType.max)
            nc.vector.tensor_scalar(
                xs_all[:, t : t + 1],
                amax,
                float(np.float32(1.0 / 127.0)),
                None,
                op0=mybir.AluOpType.mult,
            )
            inv = spool.tile([P, 1], F32, name=f"inv{t}", tag=f"inv{t}")
            nc.vector.reciprocal(inv, amax)
            nc.vector.tensor_scalar(inv, inv, QMAX, None, op0=mybir.AluOpType.mult)

            y = ypool.tile([P, K], FP16, name=f"y{t}", tag="y")
            for h in range(2):
                hs = slice(h * KH, (h + 1) * KH)
                # y = fp16(x*inv + 1536) on the scalar engine: the fp16
                # conversion RNE-rounds to integer. The GEMM runs directly on
                # the offset values; the eviction subtracts 1536*rowsum(w).
                nc.scalar.activation(
                    y[:, hs],
                    xt[:, hs],
                    mybir.ActivationFunctionType.Copy,
                    bias=MAGIC16,
                    scale=inv,
                )
                if warmup and t < 2:
                    nc.tensor.matmul(
                        wu_ps, lhsT=ident_f32, rhs=xt[:, h * KH : h * KH + 512]
                    )
                if warmup and (t < 2 or (t == 2 and h == 0)):
                    nc.tensor.matmul(
                        wu_ps, lhsT=ident_fp16, rhs=y[:, h * KH : h * KH + 512]
                    )
                # half-tile transpose on the DMA xbar (Sync queue):
                # [tok, (kc ki)] -> [ki, kc, tok]
                nc.sync.dma_start(
                    qT[:, t, h * KCH : (h + 1) * KCH, :], y[:, hs], transpose=True
                )
                if warmup and t < 2:
                    nc.tensor.matmul(
                        wu_ps[:, :P], lhsT=ident_fp16, rhs=qT[:, t, h * KCH, :]
                    )

        xs_ps = pt_pool.tile([2, 2 * P], F32, name="xs_ps", tag="xs_ps")

        def xs_pair(pair):
            # x_scale for token tiles (2*pair, 2*pair+1): PE-transpose to a
            # [t, tok] layout (contiguous 512B dram rows), round-trip through
            # DRAM on the Scalar queue to broadcast [tok] across partitions.
            ts = slice(2 * pair, 2 * pair + 2)
            cols = slice(pair * P, (pair + 1) * P)
            nc.tensor.transpose(xs_ps[:, cols], xs_all[:, ts], ident_f32)
            xs_row = spool.tile([2, P], F32, name=f"xs_row{pair}", tag=f"xs_row{pair}")
            nc.vector.tensor_copy(xs_row, xs_ps[:, cols])
            nc.sync.dma_start(
                xs_scratch.rearrange("(t p) -> t p", p=P)[ts, :], xs_row
            )
            nc.sync.dma_start(
                xsb[:, pair * SH : (pair + 1) * SH],
                xs_scratch[None, pair * SH : (pair + 1) * SH].to_broadcast((P, SH)),
            )

        def gemm_tile(nt, half):
            # half=None: full 512-token width; 0/1: 256-token half
            if nt in wtiles:
                wtile = wtiles[nt]
            else:
                wtile = wpool.tile([P, KC, P], FP16, name=f"wt{nt}", tag="wtile")
                nc.gpsimd.dma_start(wtile, wt[nt])
                wtiles[nt] = wtile
            if half is None:
                ps = ps_pool.tile([P, S_C], F32, name=f"ps{nt}", tag="ps")
                rhs_t = slice(0, TT)
                cols = slice(0, S_C)
                xs_in = xsb
            else:
                ps = ph_pool.tile([P, SH], F32, name=f"ph{nt}_{half}", tag="ph")
                rhs_t = slice(2 * half, 2 * half + 2)
                cols = slice(half * SH, (half + 1) * SH)
                xs_in = xsb[:, cols]
            for kc in range(KC):
                nc.tensor.matmul(
                    ps,
                    lhsT=wtile[:, kc, :],
                    rhs=qT[:, rhs_t, kc, :],
                    start=(kc == 0),
                    stop=(kc == KC - 1),
                )
            out_sb = opool.tile([P, S_C], F32, name=f"o{nt}_{half}", tag="out_sb")
            # acc = sum_k (q+1536)*w; (acc - 1536*rowsum_n) * ws_n * xs_tok
            nc.vector.tensor_scalar(
                out_sb[:, cols],
                ps,
                wsb_sb[:, NT + nt : NT + nt + 1],
                wsb_sb[:, nt : nt + 1],
                op0=mybir.AluOpType.add,
                op1=mybir.AluOpType.mult,
            )
            nc.vector.tensor_tensor(
                out=out_sb[:, cols], in0=out_sb[:, cols], in1=xs_in,
                op=mybir.AluOpType.mult,
            )
            nc.sync.dma_start(outT_t[nt][:, cols], out_sb[:, cols])

        # DMA issue order matters: HW-queue DMAs round-robin over 8 serial
        # semaphore lanes in issue order, so a DMA can end up waiting on the
        # DMA 8 slots earlier. Keep the order (x*8, tr-tile0/1*4, w0-2,
        # xs0, tr-tile2/3*4 interleaved, w3-5, xs1) so every lane pairing is
        # early-completing -> early-gated.
        quant_tile(0)
        quant_tile(1)
        for nt in range(3):
            preload_w(nt, amaxes[2])
        xs_pair(0)
        quant_tile(2)
        quant_tile(3)
        for nt in range(3, WBUFS):
            preload_w(nt, amaxes[3])

        # ---- Phase 2: streamed weights-stationary GEMM + fused dequant ----
        # leading tiles at half width (tokens 0-255) overlap the tail of the
        # quant pipeline; their other half + the rest run at full width.
        for nt in range(NT_EARLY):
            gemm_tile(nt, half=0)
        xs_pair(1)
        for nt in range(NT_EARLY):
            gemm_tile(nt, half=1)
        for nt in range(NT_EARLY, NT):
            gemm_tile(nt, half=None)

    return nc


def pack_inputs(input_tensor, weight, weight_scale, S_C, K, N):
    """Host-side prep: shard x, pack weights to bf16 per-nt SBUF layout."""
    KC = K // P
    NT = N // P
    x = np.ascontiguousarray(input_tensor.reshape(-1, K))  # [S, K]
    w16 = weight.astype(np.float16)  # [N, K], int8 values exact
    # pack[nt, p, kc, j] = w[nt*128 + j, kc*128 + p]
    wt = np.ascontiguousarray(w16.reshape(NT, P, KC, P).transpose(0, 3, 2, 1))
    ws = weight_scale.reshape(NT, P).T.astype(np.float32)  # [P, NT]
    # -1536 * rowsum(w): 3*rs*2^9 with |3*rs| < 2^24, exact in f32
    rs = weight.astype(np.int64).sum(axis=1).astype(np.float32)
    wb = (np.float32(-1536.0) * rs).reshape(NT, P).T
    wsb = np.ascontiguousarray(np.concatenate([ws, wb], axis=1))  # [P, 2*NT]
    return x, wt, wsb


@lru_cache(maxsize=2)
def _compiled_nc(S_C, K, N, warmup):
    return build_nc(S_C, K, N, warmup=warmup)


def run(input_tensor, weight, weight_scale, n_cores=NCORES, trace=False,
        exact_divide=True, warmup=True):
    Sfull, K = input_tensor.shape[-2], input_tensor.shape[-1]
    N = weight.shape[0]
    S_C = Sfull // n_cores
    x, wt, wsb = pack_inputs(input_tensor, weight, weight_scale, S_C, K, N)
    nc = _compiled_nc(S_C, K, N, warmup)
    in_maps = [
        {"x": np.ascontiguousarray(x[c * S_C : (c + 1) * S_C]),
         "wt": wt, "wsb": wsb}
        for c in range(n_cores)
    ]
    res = run_bass_kernel_spmd(nc, in_maps, core_ids=list(range(n_cores)), trace=trace)
    out = np.empty((Sfull, N), np.float32)
    for c in range(n_cores):
        out[c * S_C : (c + 1) * S_C] = res.results[c]["outT"].T
    return out[None], res


def kernel(input_tensor, weight, weight_scale):
    out, _ = run(
        np.asarray(input_tensor), np.asarray(weight), np.asarray(weight_scale)
    )
    return out

